# revision 46
# baseline (speedup 1.0000x reference)
"""Multi-head causal attention (B=2, S=2048, D=1024, H=16) on 8 TRN2 NeuronCores.

Sharding: core c -> batch c//4, head-quarter c%4 (4 heads = 256 head dims).
Each core runs the full pipeline for its (batch, 4 heads):
  QKV projections -> causal softmax(QK^T/8) -> PV -> partial out-projection.
Host pre-transposes x / weight shards (so every DMA is contiguous) and
sums the 4 row-sharded out-projection partials per batch + bias.

All matmul data is bf16: fp32 moving operands stream at 2 cycles/column on
the PE while bf16 streams at 1 (PSUM accumulation stays fp32). Softmax
skips max-subtraction (scores bounded ~ +-4). Causality: fully-masked
k-tiles are skipped, diagonal tiles compute only the valid [coff:] column
range (no zero-fill needed) plus one triangular multiply on GpSimd.

Engine budget: the attention inner loop is paced by ScalarE's exp
(~1.15us per merged [128,1024] head-pair tile vs ~0.85us of PE matmul),
so all other PE work - next chunk's QKV projections and the previous
chunk's out-projection - is interleaved into the attention instruction
stream as fillers to absorb the PE idle. PSUM->SBUF copies go 3:1 to
VectorE over ScalarE to keep ScalarE on exp.
"""

import sys

import numpy as np

if "/opt/trn_rl_repo" not in sys.path:
    sys.path.insert(0, "/opt/trn_rl_repo")

import ml_dtypes

import concourse.bass as bass
import concourse.mybir as mybir
import concourse.tile as tile
from concourse.bass import ts
from concourse.bass_utils import run_bass_kernel_spmd

P = 128          # partitions
S = 2048         # sequence length
DD = 1024        # model dim
DC = DD // P     # d-model chunks (8)
E = 256          # head dims per core (4 heads x 64)
H4 = 4           # heads per core
HD = 64
NQ = 4           # q chunks of 512
QC = 512
KT = S // P      # k tiles (16)
FD = 512         # matmul free dim

F32 = mybir.dt.float32
BF = mybir.dt.bfloat16
EXP = mybir.ActivationFunctionType.Exp
LN = mybir.ActivationFunctionType.Ln
MUL = mybir.AluOpType.mult
BF_NP = ml_dtypes.bfloat16


def _emit(tc, nc, xT_d, wq_d, wk_d, wv_d, wo_d, tri_d, sel_d, out_d):
    with (
        tc.tile_pool(name="const", bufs=1) as const,
        tc.tile_pool(name="attn", bufs=4) as attn_pool,
        tc.tile_pool(name="small", bufs=2) as small,
        tc.tile_pool(name="pvst", bufs=3) as pvst,
        tc.tile_pool(name="ostage", bufs=3) as ostage,
        tc.tile_pool(name="pmm", bufs=2, space="PSUM") as pmm,    # 2 banks
        tc.tile_pool(name="pacc", bufs=2, space="PSUM") as pacc,  # 2 banks
        tc.tile_pool(name="psc", bufs=2, space="PSUM") as psc,    # 4 banks
    ):
        xT = const.tile([P, DC, S], BF)
        sel = const.tile([97, 2, P], BF)
        wq = const.tile([P, DC, E], BF)
        wk = const.tile([P, DC, E], BF)
        wv = const.tile([P, DC, E], BF)
        wo = const.tile([P, 2, DD], BF)
        tri2 = const.tile([P, 2, P], BF)
        qT = const.tile([P, 2, S], BF)
        kT = const.tile([P, 2, S], BF)
        vS = const.tile([P, KT, H4, HD + 1], BF)
        cT = const.tile([P, 2, S], BF)

        # weight / mask loads first (small, needed early); x chunk j=0 in
        # half-blocks so its DMA spreads over 16 queues (faster startup).
        # Inputs issue from the GpSimd queue: its DMA sequencing cost is
        # ~25ns/instruction vs ~565ns on Sync, so loads hit the wire fast
        # (output stores stay on the otherwise-idle Sync engine).
        nc.gpsimd.dma_start(wq[:], wq_d[:])
        nc.sync.dma_start(tri2[:], tri_d[:])
        nc.sync.dma_start(sel[:], sel_d[:])
        nc.scalar.dma_start(wk[:], wk_d[:])
        nc.scalar.dma_start(wv[:], wv_d[:])
        # x chunk 0 spread across three engine DGE queues: DMA *issue* costs
        # ~0.6us per instruction per queue, so 16 serial issues on one queue
        # would add ~10us before the first projection can start
        qs = [nc.gpsimd, nc.sync, nc.scalar]
        for c in range(DC):
            for h2 in range(2):
                sl = slice(h2 * 256, (h2 + 1) * 256)
                qs[(2 * c + h2) % 3].dma_start(xT[:, c, sl], xT_d[:, c, sl])
        for j in range(1, NQ):
            for c in range(DC):
                eng = nc.gpsimd if c % 2 else nc.sync
                eng.dma_start(xT[:, c, ts(j, QC)], xT_d[:, c, ts(j, QC)])
        nc.gpsimd.dma_start(wo[:], wo_d[:])

        # constants (memset in f32, copy-cast to bf16)
        ones4 = const.tile([P, H4], F32)
        nc.vector.memset(ones4[:], 1.0)
        ones4b = const.tile([P, H4], BF)
        nc.vector.tensor_copy(ones4b[:], ones4[:])
        # ones column of V_ext (row sums of exp-scores come out of the PV matmul)
        for i in range(KT):
            nc.vector.tensor_copy(vS[:, i, :, HD], ones4b[:])

        def psum_copy(dst, src, eng="v"):
            # PSUM->SBUF copies routed explicitly: VectorE during attention
            # (ScalarE must stream exp), ScalarE in the normalize windows
            # (where it is otherwise idle and VectorE carries filler copies)
            if eng == "s":
                nc.scalar.copy(dst, src)
            else:
                nc.vector.tensor_copy(dst, src)

        def qk_gen(j):
            # ---- Q/K projections for n-chunk j; one PE matmul per pull ----
            for w_s, dst in ((wq, qT), (wk, kT)):
                for et in range(2):
                    ps = pmm.tile([P, FD], F32, tag="mm", name="ps_proj")
                    for c in range(DC):
                        nc.tensor.matmul(
                            ps[:],
                            lhsT=w_s[:, c, ts(et, P)],
                            rhs=xT[:, c, ts(j, QC)],
                            start=(c == 0),
                            stop=(c == DC - 1),
                        )
                        if c == DC - 1:
                            psum_copy(dst[:, et, ts(j, QC)], ps[:])
                        yield

        def v_gen(j):
            # ---- V projection for n-chunk j ----
            for nt in range(4 * j, 4 * j + 4):
                psv = pmm.tile([P, FD], F32, tag="mm", name="ps_v")
                for c in range(DC):
                    nc.tensor.matmul(
                        psv[:, :E],
                        lhsT=xT[:, c, ts(nt, P)],
                        rhs=wv[:, c, :],
                        start=(c == 0),
                        stop=(c == DC - 1),
                    )
                    if c == DC - 1:
                        for h in range(H4):
                            psum_copy(
                                vS[:, nt, h, 0:HD],
                                psv[:, h * HD:(h + 1) * HD],
                            )
                    yield

        def outproj_gen(j):
            # ---- out-projection for finished n-chunk j ----
            for nt in range(4 * j, 4 * j + 4):
                for fc in range(2):
                    po = pmm.tile([P, FD], F32, tag="mm", name="ps_out")
                    for c in range(2):
                        nc.tensor.matmul(
                            po[:],
                            lhsT=cT[:, c, ts(nt, P)],
                            rhs=wo[:, c, ts(fc, FD)],
                            start=(c == 0),
                            stop=(c == 1),
                        )
                        if c == 1:
                            ob = ostage.tile([P, FD], BF, tag="ob", name="ob")
                            psum_copy(ob[:], po[:])
                            # two half-stores on separate queues (short tail)
                            for q2 in range(2):
                                lo = fc * FD + q2 * 256
                                nc.sync.dma_start(
                                    out_d[ts(nt, P), lo:lo + 256],
                                    ob[:, q2 * 256:(q2 + 1) * 256],
                                )
                        yield

        def chain(*gens):
            for g in gens:
                yield from g

        npulled = [0]

        def pull(fillers, n, cap=None):
            for _ in range(n):
                if cap is not None and npulled[0] >= cap:
                    return
                if next(fillers, "done") == "done":
                    return
                npulled[0] += 1

        # warm-up: QKV for chunk 0 emitted directly (nothing to overlap yet)
        for _ in chain(qk_gen(0), v_gen(0)):
            pass

        for j in range(NQ):
            # fillers: PE work interleaved into this chunk's attention to
            # absorb PE idle while ScalarE streams exp. V for the last chunk
            # is deferred into its own attention (front-loaded: vS tile nt is
            # first read at i-tile nt, midway through) to balance the filler
            # supply against each chunk's exp-paced idle.
            gens = []
            nfill = 0
            if j == NQ - 1:
                gens.append(v_gen(j))
                nfill += 32
            if j > 0:
                gens.append(outproj_gen(j - 1))
                nfill += 16
            if j + 1 < NQ:
                gens.append(qk_gen(j + 1))
                nfill += 32
                if j + 1 < NQ - 1:
                    gens.append(v_gen(j + 1))
                    nfill += 32
            fillers = chain(*gens)
            # hold back ~3.5us of PE work to cover the rowsum chain latency
            # between the last PV and the bc matmuls
            reserve = min(nfill, 16)
            ntiles = 2 * 4 * (j + 1)
            per_tile = -(-(nfill - reserve) // ntiles)  # ceil
            if j == NQ - 1:
                # front-load so the deferred V tiles (vS[nt] first read by
                # the PV matmul at i-tile nt) are all emitted before any
                # reader: 32 pulls at 4/tile -> done by tile 8 < 12
                per_tile = 4
            npulled[0] = 0
            inline_cap = nfill - reserve

            # ---- attention for q-chunk j, heads processed in pairs ----
            nk = 4 * (j + 1)
            # rowsums parked at partitions 0/32/64/96 (engine ops need
            # 32-aligned base partitions); rows in between stay at the 1.0
            # memset so the batched reciprocal/cast read defined values
            rs = small.tile([97, QC], F32, tag="rs", name="rs")
            nc.vector.memset(rs[:], 1.0)
            pv2s = []
            for hp in range(2):
                h0, h1 = 2 * hp, 2 * hp + 1
                pvs = [
                    pacc.tile([HD + 1, QC], F32, tag="pv", name=f"pv{h}")
                    for h in (h0, h1)
                ]
                for i in range(nk):
                    # diagonal tiles only compute the causally-valid columns
                    coff = max(0, P * (i - 4 * j))
                    # both heads' scores into one 2-bank PSUM tile
                    sc2 = psc.tile([P, 2, QC], F32, tag="sc", name="sc")
                    for hh, h in enumerate((h0, h1)):
                        po = HD * (h % 2)
                        nc.tensor.matmul(
                            sc2[:, hh, coff:QC],
                            lhsT=kT[po:po + HD, hp, ts(i, P)],
                            rhs=qT[po:po + HD, hp, j * QC + coff:(j + 1) * QC],
                            start=True,
                            stop=True,
                        )
                    pull(fillers, per_tile, cap=inline_cap)
                    at2 = attn_pool.tile([P, 2, QC], BF, tag="at", name="at")
                    nc.scalar.activation(
                        at2[:, :, coff:QC], sc2[:, :, coff:QC], EXP
                    )
                    if P * (i - 4 * j) >= 0:
                        # triangular mask for the diagonal 128 columns
                        nc.gpsimd.tensor_tensor(
                            at2[:, :, coff:coff + P],
                            at2[:, :, coff:coff + P],
                            tri2[:],
                            MUL,
                        )
                    for hh, h in enumerate((h0, h1)):
                        nc.tensor.matmul(
                            pvs[hh][:, coff:QC],
                            lhsT=vS[:, i, h, :],
                            rhs=at2[:, hh, coff:QC],
                            start=(i == 0),
                            stop=(i == nk - 1),
                        )
                # drain PSUM accumulators to SBUF (frees banks for next pair):
                # unnormalized context for both heads packed into 128
                # partitions, rowsums gathered into rs rows at 0/32/64/96
                pv2 = pvst.tile([P, QC], F32, tag="pv2", name="pv2")
                pv2s.append(pv2)
                # hp0 drains on VectorE (mid-attention, ScalarE busy with
                # exp); hp1 drains on ScalarE (its exp stream just ended)
                de = "v" if hp == 0 else "s"
                for hh, h in enumerate((h0, h1)):
                    psum_copy(
                        pv2[HD * hh:HD * (hh + 1), :], pvs[hh][0:HD, :], de
                    )
                    r = 32 * (2 * hp + hh)
                    psum_copy(rs[r:r + 1, :], pvs[hh][HD:HD + 1, :], de)
            # 1/rowsum for all 4 heads at once as exp(-ln(s)) on ScalarE —
            # idle here, emits bf16 directly, and keeps VectorE free so the
            # reserved fillers' PSUM drains can flow during this chain
            rs_l = small.tile([97, QC], F32, tag="rsl", name="rs_l")
            nc.scalar.activation(rs_l[:], rs[:], LN)
            rs_b = small.tile([97, QC], BF, tag="rsb", name="rs_b")
            nc.scalar.activation(rs_b[:], rs_l[:], EXP, scale=-1.0)
            for _ in fillers:  # reserved fillers cover the chain latency
                pass
            for hp in range(2):
                # selector matmul broadcasts each head's 1/rowsum across its
                # 64 context partitions: bc[m, q] = rs_b[32*(2hp + m//64), q]
                bc_ps = pmm.tile([P, FD], F32, tag="mm", name="bc_ps")
                nc.tensor.matmul(
                    bc_ps[:], lhsT=sel[:, hp, :], rhs=rs_b[:],
                    start=True, stop=True,
                )
                nc.vector.tensor_tensor(
                    cT[:, hp, ts(j, QC)], pv2s[hp][:], bc_ps[:], MUL
                )

        # tail: out-projection for the last chunk
        for _ in outproj_gen(NQ - 1):
            pass


def _split_multi_waits(nc):
    """The TRN2 instruction encoding carries ONE sync-wait slot; this walrus
    build rejects instructions with more. Hoist extra waits onto standalone
    EventSemaphore instructions immediately before (same engine queue, same
    semantics)."""
    n = 0
    for f in nc.m.functions:
        for b in f.blocks:
            out = []
            for i in list(b.instructions):
                si = i.sync_info
                if si is not None and len(si.on_wait) > 1:
                    waits = list(si.on_wait)
                    for w in waits[:-1]:
                        n += 1
                        out.append(
                            mybir.InstEventSemaphore(
                                name=f"I-wsplit{n}",
                                engine=i.engine,
                                ins=[],
                                outs=[],
                                sync_info=mybir.SyncInfo(on_wait=[w], on_update=[]),
                            )
                        )
                    i.sync_info = mybir.SyncInfo(
                        on_wait=[waits[-1]], on_update=list(si.on_update)
                    )
                out.append(i)
            b.instructions = out


def build_nc(split_waits=True):
    nc = bass.Bass("TRN2", target_bir_lowering=False, debug=False)
    xT_d = nc.dram_tensor("xT", [P, DC, S], BF, kind="ExternalInput").ap()
    wq_d = nc.dram_tensor("wqT", [P, DC, E], BF, kind="ExternalInput").ap()
    wk_d = nc.dram_tensor("wkT", [P, DC, E], BF, kind="ExternalInput").ap()
    wv_d = nc.dram_tensor("wvT", [P, DC, E], BF, kind="ExternalInput").ap()
    wo_d = nc.dram_tensor("woT", [P, 2, DD], BF, kind="ExternalInput").ap()
    tri_d = nc.dram_tensor("tri", [P, 2, P], BF, kind="ExternalInput").ap()
    sel_d = nc.dram_tensor("sel", [97, 2, P], BF, kind="ExternalInput").ap()
    out_d = nc.dram_tensor("out", [S, DD], BF, kind="ExternalOutput").ap()
    with tile.TileContext(nc) as tc:
        _emit(tc, nc, xT_d, wq_d, wk_d, wv_d, wo_d, tri_d, sel_d, out_d)
    if split_waits:
        _split_multi_waits(nc)
    return nc


def _strip(a, chunks):
    """[D, N] -> [128, D//128, N] with partition-major layout, contiguous."""
    d, n = a.shape
    return np.ascontiguousarray(
        a.reshape(chunks, P, n).transpose(1, 0, 2)
    ).astype(BF_NP)


def make_in_maps(x, Wq, Wk, Wv, Wo):
    tri = np.triu(np.ones((P, P), np.float32))
    tri2 = np.ascontiguousarray(
        np.repeat(tri[:, None, :], 2, axis=1)
    ).astype(BF_NP)
    sel = np.zeros((97, 2, P), np.float32)
    for hp in range(2):
        for m in range(P):
            sel[32 * (2 * hp + m // HD), hp, m] = 1.0
    sel = sel.astype(BF_NP)
    in_maps = []
    for c in range(8):
        b, g = c // 4, c % 4
        sl = slice(E * g, E * (g + 1))
        in_maps.append(
            {
                "xT": _strip(x[b].T.astype(np.float32), DC),
                "wqT": _strip((Wq[sl, :] * 0.125).T.astype(np.float32), DC),
                "wkT": _strip(Wk[sl, :].T.astype(np.float32), DC),
                "wvT": _strip(Wv[sl, :].T.astype(np.float32), DC),
                "woT": _strip(Wo[:, sl].T.astype(np.float32), 2),
                "tri": tri2,
                "sel": sel,
            }
        )
    return in_maps


def kernel(x, Wq, Wk, Wv, Wo, bo, _run_kwargs=None):
    x, Wq, Wk, Wv, Wo, bo = (
        np.asarray(a, dtype=np.float32) for a in (x, Wq, Wk, Wv, Wo, bo)
    )
    nc = build_nc()
    in_maps = make_in_maps(x, Wq, Wk, Wv, Wo)
    res = run_bass_kernel_spmd(
        nc, in_maps, core_ids=list(range(8)), **(_run_kwargs or {})
    )
    out = np.zeros((2, S, DD), dtype=np.float32)
    for c in range(8):
        out[c // 4] += res.results[c]["out"].astype(np.float32)
    out += bo[None, None, :]
    if _run_kwargs:
        kernel.last_results = res
    return out


# revision 50
# speedup vs baseline: 1.1069x; 1.1069x over previous
"""Multi-head causal attention (B=2, S=2048, D=1024, H=16) on 8 TRN2 NeuronCores.

Sharding: core c -> batch c//4, head-quarter c%4 (4 heads = 256 head dims).
Each core runs the full pipeline for its (batch, 4 heads):
  QKV projections -> causal softmax(QK^T/8) -> PV -> partial out-projection.
Host pre-transposes x / weight shards (so every DMA is contiguous) and
sums the 4 row-sharded out-projection partials per batch + bias.

All matmul data is bf16: fp32 moving operands stream at 2 cycles/column on
the PE while bf16 streams at 1 (PSUM accumulation stays fp32). Softmax
skips max-subtraction (scores bounded ~ +-4). Causality: fully-masked
k-tiles are skipped, diagonal tiles compute only the valid [coff:] column
range (no zero-fill needed) plus one triangular multiply on GpSimd.

Engine budget: the attention inner loop is paced by ScalarE's exp
(~1.15us per merged [128,1024] head-pair tile vs ~0.85us of PE matmul),
so all other PE work - next chunk's QKV projections and the previous
chunk's out-projection - is interleaved into the attention instruction
stream as fillers to absorb the PE idle. Normalization runs per head
pair: rowsums gather on ScalarE, 1/s = exp(-ln(s)) on ScalarE (same ACT
table set as the softmax exp), broadcast via a selector matmul; the
first pair's chain hides under the second pair's attention.
"""

import sys

import numpy as np

if "/opt/trn_rl_repo" not in sys.path:
    sys.path.insert(0, "/opt/trn_rl_repo")

import ml_dtypes

import concourse.bass as bass
import concourse.mybir as mybir
import concourse.tile as tile
from concourse.bass import ts
from concourse.bass_utils import run_bass_kernel_spmd

P = 128          # partitions
S = 2048         # sequence length
DD = 1024        # model dim
DC = DD // P     # d-model chunks (8)
E = 256          # head dims per core (4 heads x 64)
H4 = 4           # heads per core
HD = 64
NQ = 4           # q chunks of 512
QC = 512
KT = S // P      # k tiles (16)
FD = 512         # matmul free dim

F32 = mybir.dt.float32
BF = mybir.dt.bfloat16
EXP = mybir.ActivationFunctionType.Exp
LN = mybir.ActivationFunctionType.Ln
MUL = mybir.AluOpType.mult
BF_NP = ml_dtypes.bfloat16


def _emit(tc, nc, xT_d, wq_d, wk_d, wv_d, wo_d, tri_d, sel_d, out_d):
    with (
        tc.tile_pool(name="const", bufs=1) as const,
        tc.tile_pool(name="attn", bufs=4) as attn_pool,
        tc.tile_pool(name="small", bufs=2) as small,
        tc.tile_pool(name="pvst", bufs=3) as pvst,
        tc.tile_pool(name="ostage", bufs=2) as ostage,
        tc.tile_pool(name="pmm", bufs=2, space="PSUM") as pmm,    # 2 banks
        tc.tile_pool(name="pacc", bufs=2, space="PSUM") as pacc,  # 2 banks
        tc.tile_pool(name="psc", bufs=2, space="PSUM") as psc,    # 4 banks
    ):
        # x is staged j-major ([P, NQ, DC, QC]) so each chunk's load is one
        # 8KB/partition fully-contiguous DMA (big descriptors, full DMA bw)
        xT = const.tile([P, NQ, DC, QC], BF)
        sel = const.tile([33, 2, P], BF)
        wq = const.tile([P, DC, E], BF)
        wk = const.tile([P, DC, E], BF)
        wv = const.tile([P, DC, E], BF)
        wo = const.tile([P, 2, DD], BF)
        tri2 = const.tile([P, 2, P], BF)
        qT = const.tile([P, 2, S], BF)
        kT = const.tile([P, 2, S], BF)
        vS = const.tile([P, KT, H4, HD + 1], BF)
        cT = const.tile([P, 2, S], BF)

        # x chunk 0 first (computation gates on it), split across the two
        # least-loaded DGE queues; weights next; later chunks follow
        nc.gpsimd.dma_start(xT[:, 0, 0:4], xT_d[:, 0, 0:4])
        nc.sync.dma_start(xT[:, 0, 4:8], xT_d[:, 0, 4:8])
        nc.scalar.dma_start(wq[:], wq_d[:])
        nc.scalar.dma_start(wk[:], wk_d[:])
        nc.scalar.dma_start(wv[:], wv_d[:])
        nc.sync.dma_start(tri2[:], tri_d[:])
        nc.sync.dma_start(sel[:], sel_d[:])
        for j in range(1, NQ):
            nc.gpsimd.dma_start(xT[:, j, 0:4], xT_d[:, j, 0:4])
            nc.sync.dma_start(xT[:, j, 4:8], xT_d[:, j, 4:8])
        nc.gpsimd.dma_start(wo[:], wo_d[:])

        # constants (memset in f32, copy-cast to bf16)
        ones4 = const.tile([P, H4], F32)
        nc.vector.memset(ones4[:], 1.0)
        ones4b = const.tile([P, H4], BF)
        nc.vector.tensor_copy(ones4b[:], ones4[:])
        # ones column of V_ext (row sums of exp-scores come out of the PV matmul)
        for i in range(KT):
            nc.vector.tensor_copy(vS[:, i, :, HD], ones4b[:])

        def psum_copy(dst, src, eng="v"):
            # PSUM->SBUF copies routed explicitly: VectorE during attention
            # (ScalarE must stream exp), ScalarE in the normalize windows
            if eng == "s":
                nc.scalar.copy(dst, src)
            else:
                nc.vector.tensor_copy(dst, src)

        def qk_gen(j):
            # ---- Q/K projections for n-chunk j; one PE matmul per pull ----
            for w_s, dst in ((wq, qT), (wk, kT)):
                for et in range(2):
                    ps = pmm.tile([P, FD], F32, tag="mm", name="ps_proj")
                    for c in range(DC):
                        nc.tensor.matmul(
                            ps[:],
                            lhsT=w_s[:, c, ts(et, P)],
                            rhs=xT[:, j, c, :],
                            start=(c == 0),
                            stop=(c == DC - 1),
                        )
                        if c == DC - 1:
                            psum_copy(dst[:, et, ts(j, QC)], ps[:])
                        yield

        def v_gen(j):
            # ---- V projection for n-chunk j ----
            for nt in range(4 * j, 4 * j + 4):
                psv = pmm.tile([P, FD], F32, tag="mm", name="ps_v")
                for c in range(DC):
                    nc.tensor.matmul(
                        psv[:, :E],
                        lhsT=xT[:, j, c, ts(nt % 4, P)],
                        rhs=wv[:, c, :],
                        start=(c == 0),
                        stop=(c == DC - 1),
                    )
                    if c == DC - 1:
                        for h in range(H4):
                            psum_copy(
                                vS[:, nt, h, 0:HD],
                                psv[:, h * HD:(h + 1) * HD],
                            )
                    yield

        def outproj_gen(j):
            # ---- out-projection for finished n-chunk j ----
            for nt in range(4 * j, 4 * j + 4):
                ob = ostage.tile([P, DD], BF, tag="ob", name="ob")
                for fc in range(2):
                    po = pmm.tile([P, FD], F32, tag="mm", name="ps_out")
                    for c in range(2):
                        nc.tensor.matmul(
                            po[:],
                            lhsT=cT[:, c, ts(nt, P)],
                            rhs=wo[:, c, ts(fc, FD)],
                            start=(c == 0),
                            stop=(c == 1),
                        )
                        if c == 1:
                            psum_copy(ob[:, ts(fc, FD)], po[:])
                            if fc == 1:
                                # one store per 128-row block: 2KB/row fully
                                # contiguous DRAM rows (max DMA efficiency)
                                nc.sync.dma_start(out_d[ts(nt, P), :], ob[:])
                        yield

        def chain(*gens):
            for g in gens:
                yield from g

        npulled = [0]

        def pull(fillers, n, cap=None):
            for _ in range(n):
                if cap is not None and npulled[0] >= cap:
                    return
                if next(fillers, "done") == "done":
                    return
                npulled[0] += 1

        # warm-up: QKV for chunk 0 emitted directly (nothing to overlap yet)
        for _ in chain(qk_gen(0), v_gen(0)):
            pass

        for j in range(NQ):
            # fillers: PE work interleaved into this chunk's attention to
            # absorb PE idle while ScalarE streams exp. V for the last chunk
            # is deferred into its own attention (front-loaded: vS tile nt is
            # first read at i-tile nt, midway through) to balance the filler
            # supply against each chunk's exp-paced idle.
            gens = []
            nfill = 0
            if j == NQ - 1:
                gens.append(v_gen(j))
                nfill += 32
            if j > 0:
                gens.append(outproj_gen(j - 1))
                nfill += 16
            if j + 1 < NQ:
                gens.append(qk_gen(j + 1))
                nfill += 32
                if j + 1 < NQ - 1:
                    gens.append(v_gen(j + 1))
                    nfill += 32
            fillers = chain(*gens)
            # hold back ~3us of PE work to cover the hp1 rowsum-chain
            # latency between the last PV and its bc matmul
            reserve = min(nfill, 14)
            ntiles = 2 * 4 * (j + 1)
            per_tile = -(-(nfill - reserve) // ntiles)  # ceil
            if j == NQ - 1:
                # front-load so the deferred V tiles (vS[nt] first read by
                # the PV matmul at i-tile nt) are all emitted before any
                # reader: 32+2 pulls at 5/tile -> done by tile 7 < 12
                per_tile = 5
            npulled[0] = 0
            inline_cap = nfill - reserve

            # ---- attention for q-chunk j, heads processed in pairs ----
            nk = 4 * (j + 1)
            pending = []  # hp0's normalize closure, run mid-hp1
            for hp in range(2):
                h0, h1 = 2 * hp, 2 * hp + 1
                pvs = [
                    pacc.tile([HD + 1, QC], F32, tag="pv", name=f"pv{h}")
                    for h in (h0, h1)
                ]
                for i in range(nk):
                    # diagonal tiles only compute the causally-valid columns
                    coff = max(0, P * (i - 4 * j))
                    # both heads' scores into one 2-bank PSUM tile
                    sc2 = psc.tile([P, 2, QC], F32, tag="sc", name="sc")
                    for hh, h in enumerate((h0, h1)):
                        po = HD * (h % 2)
                        nc.tensor.matmul(
                            sc2[:, hh, coff:QC],
                            lhsT=kT[po:po + HD, hp, ts(i, P)],
                            rhs=qT[po:po + HD, hp, j * QC + coff:(j + 1) * QC],
                            start=True,
                            stop=True,
                        )
                    pull(fillers, per_tile, cap=inline_cap)
                    if pending and i >= min(6, nk - 1):
                        pending.pop()()
                    at2 = attn_pool.tile([P, 2, QC], BF, tag="at", name="at")
                    nc.scalar.activation(
                        at2[:, :, coff:QC], sc2[:, :, coff:QC], EXP
                    )
                    if P * (i - 4 * j) >= 0:
                        # triangular mask for the diagonal 128 columns
                        nc.gpsimd.tensor_tensor(
                            at2[:, :, coff:coff + P],
                            at2[:, :, coff:coff + P],
                            tri2[:],
                            MUL,
                        )
                    for hh, h in enumerate((h0, h1)):
                        nc.tensor.matmul(
                            pvs[hh][:, coff:QC],
                            lhsT=vS[:, i, h, :],
                            rhs=at2[:, hh, coff:QC],
                            start=(i == 0),
                            stop=(i == nk - 1),
                        )
                # drain the pair's PSUM accumulators and normalize:
                # unnormalized context (both heads) packed into 128
                # partitions on VectorE; rowsums gathered to partitions
                # 0/32 of a [33, QC] tile on ScalarE, then 1/s =
                # exp(-ln(s)) on ScalarE (bf16 out, no extra cast)
                pv2 = pvst.tile([P, QC], F32, tag="pv2", name="pv2")
                rs = small.tile([33, QC], F32, tag="rs", name="rs")
                nc.vector.memset(rs[:], 1.0)
                # hp0's chain runs on VectorE (slack mid-attention; keeps
                # the ScalarE exp stream for hp1 unbroken); hp1's runs on
                # ScalarE in its post-exp idle window as 1/s = exp(-ln(s))
                de = "v" if hp == 0 else "s"
                for hh, h in enumerate((h0, h1)):
                    psum_copy(pv2[HD * hh:HD * (hh + 1), :], pvs[hh][0:HD, :])
                    psum_copy(rs[32 * hh:32 * hh + 1, :],
                              pvs[hh][HD:HD + 1, :], de)
                rs_b = small.tile([33, QC], BF, tag="rsb", name="rs_b")
                if hp == 0:
                    rs_r = small.tile([33, QC], F32, tag="rsr", name="rs_r")
                    nc.vector.reciprocal(rs_r[:], rs[:])
                    nc.vector.tensor_copy(rs_b[:], rs_r[:])
                else:
                    rs_l = small.tile([33, QC], F32, tag="rsl", name="rs_l")
                    nc.scalar.activation(rs_l[:], rs[:], LN)
                    nc.scalar.activation(rs_b[:], rs_l[:], EXP, scale=-1.0)

                def normalize(hp=hp, pv2=pv2, rs_b=rs_b):
                    # selector matmul broadcasts each head's 1/rowsum
                    # across its 64 context partitions:
                    # bc[m, q] = rs_b[32*(m//64), q]
                    bc_ps = pmm.tile([P, FD], F32, tag="mm", name="bc_ps")
                    nc.tensor.matmul(
                        bc_ps[:], lhsT=sel[:, hp, :], rhs=rs_b[:],
                        start=True, stop=True,
                    )
                    nc.vector.tensor_tensor(
                        cT[:, hp, ts(j, QC)], pv2[:], bc_ps[:], MUL
                    )

                if hp == 0:
                    # run after a couple of hp1 tiles so the PE does not
                    # sit on the bc matmul waiting for this chain
                    pending.append(normalize)
                else:
                    pull(fillers, nfill)  # reserved fillers cover the chain
                    normalize()

        # tail: out-projection for the last chunk
        for _ in outproj_gen(NQ - 1):
            pass


def _split_multi_waits(nc):
    """The TRN2 instruction encoding carries ONE sync-wait slot; this walrus
    build rejects instructions with more. Hoist extra waits onto standalone
    EventSemaphore instructions immediately before (same engine queue, same
    semantics)."""
    n = 0
    for f in nc.m.functions:
        for b in f.blocks:
            out = []
            for i in list(b.instructions):
                si = i.sync_info
                if si is not None and len(si.on_wait) > 1:
                    waits = list(si.on_wait)
                    for w in waits[:-1]:
                        n += 1
                        out.append(
                            mybir.InstEventSemaphore(
                                name=f"I-wsplit{n}",
                                engine=i.engine,
                                ins=[],
                                outs=[],
                                sync_info=mybir.SyncInfo(on_wait=[w], on_update=[]),
                            )
                        )
                    i.sync_info = mybir.SyncInfo(
                        on_wait=[waits[-1]], on_update=list(si.on_update)
                    )
                out.append(i)
            b.instructions = out


def build_nc(split_waits=True):
    nc = bass.Bass("TRN2", target_bir_lowering=False, debug=False)
    xT_d = nc.dram_tensor("xT", [P, NQ, DC, QC], BF, kind="ExternalInput").ap()
    wq_d = nc.dram_tensor("wqT", [P, DC, E], BF, kind="ExternalInput").ap()
    wk_d = nc.dram_tensor("wkT", [P, DC, E], BF, kind="ExternalInput").ap()
    wv_d = nc.dram_tensor("wvT", [P, DC, E], BF, kind="ExternalInput").ap()
    wo_d = nc.dram_tensor("woT", [P, 2, DD], BF, kind="ExternalInput").ap()
    tri_d = nc.dram_tensor("tri", [P, 2, P], BF, kind="ExternalInput").ap()
    sel_d = nc.dram_tensor("sel", [33, 2, P], BF, kind="ExternalInput").ap()
    out_d = nc.dram_tensor("out", [S, DD], BF, kind="ExternalOutput").ap()
    with tile.TileContext(nc) as tc:
        _emit(tc, nc, xT_d, wq_d, wk_d, wv_d, wo_d, tri_d, sel_d, out_d)
    if split_waits:
        _split_multi_waits(nc)
    return nc


def _strip(a, chunks):
    """[D, N] -> [128, D//128, N] with partition-major layout, contiguous."""
    d, n = a.shape
    return np.ascontiguousarray(
        a.reshape(chunks, P, n).transpose(1, 0, 2)
    ).astype(BF_NP)


def make_in_maps(x, Wq, Wk, Wv, Wo):
    tri = np.triu(np.ones((P, P), np.float32))
    tri2 = np.ascontiguousarray(
        np.repeat(tri[:, None, :], 2, axis=1)
    ).astype(BF_NP)
    sel = np.zeros((33, 2, P), np.float32)
    for hp in range(2):
        for m in range(P):
            sel[32 * (m // HD), hp, m] = 1.0
    sel = sel.astype(BF_NP)
    in_maps = []
    for c in range(8):
        b, g = c // 4, c % 4
        sl = slice(E * g, E * (g + 1))
        xs = _strip(x[b].T.astype(np.float32), DC)          # [P, DC, S]
        xs = np.ascontiguousarray(
            xs.reshape(P, DC, NQ, QC).transpose(0, 2, 1, 3)  # [P, NQ, DC, QC]
        )
        in_maps.append(
            {
                "xT": xs,
                "wqT": _strip((Wq[sl, :] * 0.125).T.astype(np.float32), DC),
                "wkT": _strip(Wk[sl, :].T.astype(np.float32), DC),
                "wvT": _strip(Wv[sl, :].T.astype(np.float32), DC),
                "woT": _strip(Wo[:, sl].T.astype(np.float32), 2),
                "tri": tri2,
                "sel": sel,
            }
        )
    return in_maps


def kernel(x, Wq, Wk, Wv, Wo, bo, _run_kwargs=None):
    x, Wq, Wk, Wv, Wo, bo = (
        np.asarray(a, dtype=np.float32) for a in (x, Wq, Wk, Wv, Wo, bo)
    )
    nc = build_nc()
    in_maps = make_in_maps(x, Wq, Wk, Wv, Wo)
    res = run_bass_kernel_spmd(
        nc, in_maps, core_ids=list(range(8)), **(_run_kwargs or {})
    )
    out = np.zeros((2, S, DD), dtype=np.float32)
    for c in range(8):
        out[c // 4] += res.results[c]["out"].astype(np.float32)
    out += bo[None, None, :]
    if _run_kwargs:
        kernel.last_results = res
    return out
